# revision 28
# baseline (speedup 1.0000x reference)
"""Trainium2 Bass kernel for nn_CTRule (temporal KG scoring model).

Computes, for each of B=1024 queries (h, r, t):
  v = f(E0[h], E1[r], time tables, rule tables)   # [B, 128] elementwise algebra
  scores = v @ E0.T                               # [B, 40000]

Distribution over the 8 NeuronCores (pair-hybrid): the two cores of pair p
process batch tiles 2p, 2p+1 against disjoint halves of the 40000-entity
axis.  Per core: gather (indirect DMA) the per-example table rows, run the
elementwise head in fp16 on VectorE (+GpSimd for the independent rule
chain), transpose v on TensorE, stream this core's E0T half through
40-chunk matmuls per tile, and write the fp16 [256, 20000] block to HBM.

Latency structure (from trace analysis of prior versions):
  * idx is DMA'd by gpsimd itself (lands ~3us; the HWDGE rings only reach
    their first issue slot at ~5-7us after library loads).
  * gathers issue back-to-back on gpsimd right after idx; tile 0's tables
    first.  Tables are host-augmented with swapped halves ([x0|x1]->[x1|x0])
    so every complex-product pair is ONE wide [P,256] fp16 multiply.
  * the head is a single-engine chain on Vector (no cross-engine ping-pong)
    except the rule cmul which GpSimd computes concurrently.
  * E0T chunk loads run on the Activation HWDGE ring from ~7us (no deps).
  * matmul chunks are 512 cols (one PSUM bank); pairs share a [P,1024] PSUM
    tile drained by one copy (Vector/Scalar alternate); every 1024-col group
    is DMA'd to HBM on the Sync ring as soon as its copy lands, so the
    ~31us write stream overlaps everything else.
All head math in fp16 (rel err ~6e-4 total vs the 2e-2 gate).  No
cross-core communication; the host reassembles the 8 blocks.
"""

import numpy as np

P = 128
B = 1024
RANK = 128
NENT = 40000
NREL = 230
NTIME = 365
CYCLE = 120
NCORES = 8
NHALF = NENT // 2        # entity columns per core = 20000
CHUNK = 512              # matmul chunk columns (= one PSUM bank of f32)
LOADCH = 2500            # E0T load-chunk columns (8 loads of 0.64MB)
OUTCH = 1024             # output DMA group columns (= one copy group)

RC_W = 4 * RANK + 2      # [E1 | E1sw | rule_C | rule_Csw | -rS | hr] = 514
TC_W = 5 * RANK          # [E4 | E4 | TM | TE | TEsw] = 640
LH_W = 2 * RANK          # [E0row | E0row-swapped] = 256

TRACE = False            # set by test harness for profiling runs
_CACHE = {}


def _build():
    import concourse.bass as bass
    import concourse.mybir as mybir
    import concourse.tile as tile
    from concourse import bacc
    from concourse.masks import make_identity

    dt = mybir.dt
    mult = mybir.AluOpType.mult
    add = mybir.AluOpType.add
    sub = mybir.AluOpType.subtract

    nc = bacc.Bacc("TRN2", target_bir_lowering=False, debug=False,
                   num_devices=NCORES)

    IDX = nc.dram_tensor("IDX", [P, 8], dt.int32, kind="ExternalInput").ap()
    E0G = nc.dram_tensor("E0G", [NENT, LH_W], dt.float16, kind="ExternalInput").ap()
    RCAT = nc.dram_tensor("RCAT", [NREL, RC_W], dt.float16, kind="ExternalInput").ap()
    TCAT = nc.dram_tensor("TCAT", [NTIME, TC_W], dt.float16, kind="ExternalInput").ap()
    E0T = nc.dram_tensor("E0T", [RANK, NHALF], dt.float16, kind="ExternalInput").ap()
    OUT = nc.dram_tensor("OUT", [2 * P, NHALF], dt.float16, kind="ExternalOutput").ap()

    with tile.TileContext(nc) as tc:
        with (
            tc.tile_pool(name="const", bufs=1) as constp,
            tc.tile_pool(name="gath", bufs=1) as gp,
            tc.tile_pool(name="ew", bufs=1) as ew,
            tc.tile_pool(name="pst", bufs=1, space="PSUM") as pst,
            tc.tile_pool(name="psm", bufs=3, space="PSUM") as psm,
        ):
            # ---- idx as the scalar ring's first issue (~7.5us packets; the
            # gpsimd SWDGE path would take ~10.5us for the same load).
            idxt = gp.tile([P, 8], dt.int32)
            nc.scalar.dma_start(idxt[:], IDX[:])

            # idx layout: cols (r0,r1, t0,t1, h0,h1, pad,pad); tile0 first
            lhsv = gp.tile([P, 2, LH_W], dt.float16, name="lhs")
            r8v = gp.tile([P, 2, RC_W], dt.float16, name="r8")
            t8v = gp.tile([P, 2, TC_W], dt.float16, name="t8")
            for j in range(2):
                for dst, src, col in ((r8v, RCAT, 0), (t8v, TCAT, 2),
                                      (lhsv, E0G, 4)):
                    nc.gpsimd.indirect_dma_start(
                        out=dst[:, j, :], out_offset=None, in_=src[:],
                        in_offset=bass.IndirectOffsetOnAxis(
                            ap=idxt[:, col + j:col + j + 1], axis=0))

            # ---- E0T half-table stream on the Activation HWDGE ring
            e0t = constp.tile([RANK, NHALF], dt.float16)
            for c0 in range(0, NHALF, LOADCH):
                nc.scalar.dma_start(e0t[:, c0:c0 + LOADCH],
                                    E0T[:, c0:c0 + LOADCH])

            ident = constp.tile([P, P], dt.float16)
            make_identity(nc, ident[:])

            def VTT(out, a, b_, op):
                nc.vector.tensor_tensor(out=out, in0=a, in1=b_, op=op)

            def GTT(out, a, b_, op):
                nc.gpsimd.tensor_tensor(out=out, in0=a, in1=b_, op=op)

            A = [ew.tile([P, RANK], dt.float16, name=f"A{j}") for j in range(2)]
            Bt = [ew.tile([P, RANK], dt.float16, name=f"B{j}") for j in range(2)]
            PB = [ew.tile([P, 2 * RANK], dt.float16, name=f"PB{j}") for j in range(2)]
            QQ = [ew.tile([P, 2 * RANK], dt.float16, name=f"QQ{j}") for j in range(2)]
            SS = [ew.tile([P, 2 * RANK], dt.float16, name=f"SS{j}") for j in range(2)]
            DD = [ew.tile([P, 2 * RANK], dt.float16, name=f"DD{j}") for j in range(2)]
            PL = [ew.tile([P, 2 * RANK], dt.float16, name=f"PL{j}") for j in range(2)]
            PT = [ew.tile([P, 2 * RANK], dt.float16, name=f"PT{j}") for j in range(2)]
            t0 = [ew.tile([P, 64], dt.float16, name=f"t0_{j}") for j in range(2)]
            t1 = [ew.tile([P, 64], dt.float16, name=f"t1_{j}") for j in range(2)]
            u0 = [ew.tile([P, 64], dt.float16, name=f"u0_{j}") for j in range(2)]
            V = [ew.tile([P, RANK], dt.float16, name=f"V{j}") for j in range(2)]

            def head_gpsimd_rule(j):
                # A = cmul(CT, RC):  PA = [CT|CT]*[RC|RCsw] then halves.
                # PA = [CT0RC0|CT1RC1 | CT0RC1|CT1RC0]
                PA = ew.tile([P, 2 * RANK], dt.float16, name=f"PA{j}")
                GTT(PA[:], t8v[:, j, 0:256], r8v[:, j, 256:512], mult)
                GTT(A[j][:, 0:64], PA[:, 0:64], PA[:, 64:128], sub)
                GTT(A[j][:, 64:128], PA[:, 128:192], PA[:, 192:256], add)

            def head_gpsimd_tail(j):
                # after Vector finishes rel_ (Bt), gpsimd computes the
                # DD/PT branch and V1's PT half while Vector does SS/PL/V0
                t8 = t8v[:, j, :]
                TM = t8[:, 256:384]
                TESW2 = t8[:, 384:640]
                Bj = Bt[j]
                GTT(DD[j][:, 0:128], Bj[:], TM, sub)
                GTT(DD[j][:, 128:256], Bj[:], TM, sub)
                GTT(PT[j][:], TESW2, DD[j][:], mult)
                # t1d = TE1D0 - TE0D1  (V1's PT half)
                GTT(t1[j][:], PT[j][:, 128:192], PT[j][:, 192:256], sub)

            def head_vector(j):
                r8 = r8v[:, j, :]
                t8 = t8v[:, j, :]
                lhs = lhsv[:, j, :]
                RELRELSW = r8[:, 0:256]
                REL = r8[:, 0:128]
                NRS = r8[:, 512:513]
                HR = r8[:, 513:514]
                CT = t8[:, 0:128]
                TM = t8[:, 256:384]
                TESW2 = t8[:, 384:640]
                LHS = lhs[:, 0:128]
                Aj, Bj = A[j], Bt[j]
                # B = lhs + cmul(REL, LHS):
                # PB = [REL|RELsw]*[L|L] = [RL0L0|RL1L1 | RL1L0|RL0L1]
                VTT(PB[j][:], RELRELSW, lhs[:, 0:256], mult)
                VTT(Bj[:, 0:64], PB[j][:, 0:64], PB[j][:, 64:128], sub)
                VTT(Bj[:, 64:128], PB[j][:, 128:192], PB[j][:, 192:256], add)
                VTT(Bj[:], Bj[:], LHS, add)
                # A = rule_branch = cmul(CT,RC) - rule_S*rel  (NRS = -rule_S)
                nc.vector.scalar_tensor_tensor(
                    out=Aj[:], in0=REL, scalar=NRS, in1=Aj[:],
                    op0=mult, op1=add)
                # A = rule_score = B + HR*(A - B); qq = [A+CT | A+CT]
                VTT(Aj[:], Aj[:], Bj[:], sub)
                nc.vector.scalar_tensor_tensor(
                    out=Aj[:], in0=Aj[:], scalar=HR, in1=Bj[:],
                    op0=mult, op1=add)
                VTT(QQ[j][:, 0:128], Aj[:], CT, add)
                VTT(QQ[j][:, 128:256], Aj[:], CT, add)
                # C = rel_ = REL + complex_mul(REL, q)
                # PC = [REL|RELsw]*[q|q] = [RL0q0|RL1q1 | RL1q0|RL0q1]
                PC = PB[j]
                VTT(PC[:], RELRELSW, QQ[j][:], mult)
                VTT(Bj[:, 0:64], PC[:, 0:64], PC[:, 64:128], add)
                VTT(Bj[:, 64:128], PC[:, 192:256], PC[:, 128:192], sub)
                VTT(Bj[:], Bj[:], REL, add)
                # SS = [S|Ssw] with S = rel_+time (DD/PT run on gpsimd)
                TM0 = t8[:, 256:320]
                TM1 = t8[:, 320:384]
                VTT(SS[j][:, 0:128], Bj[:], TM, add)
                VTT(SS[j][:, 128:192], Bj[:, 64:128], TM1, add)
                VTT(SS[j][:, 192:256], Bj[:, 0:64], TM0, add)
                # PL = [L|L]*[S|Ssw] = [L0S0|L1S1 | L0S1|L1S0]
                VTT(PL[j][:], lhs[:, 0:256], SS[j][:], mult)

            def head_vector_b(j):
                # V0 = (L0S0 - L1S1) + (TE0D0 + TE1D1)
                VTT(t0[j][:], PL[j][:, 0:64], PL[j][:, 64:128], sub)
                VTT(u0[j][:], PT[j][:, 0:64], PT[j][:, 64:128], add)
                VTT(V[j][:, 0:64], t0[j][:], u0[j][:], add)
                # V1 = (L0S1 + L1S0) + (TE1D0 - TE0D1)
                VTT(t0[j][:], PL[j][:, 128:192], PL[j][:, 192:256], add)
                VTT(V[j][:, 64:128], t0[j][:], t1[j][:], add)

            vts = []

            def finish_vt(j):
                vt_ps = pst.tile([P, P], dt.float16, space="PSUM", tag="vtps")
                nc.tensor.transpose(out=vt_ps[:], in_=V[j][:], identity=ident[:])
                vt = constp.tile([P, P], dt.float16, name=f"vt{j}")
                nc.scalar.copy(out=vt[:], in_=vt_ps[:])
                vts.append(vt)

            head_gpsimd_rule(0)
            head_gpsimd_rule(1)
            head_vector(0)
            head_gpsimd_tail(0)
            head_vector_b(0)
            finish_vt(0)
            head_vector(1)
            head_gpsimd_tail(1)
            head_vector_b(1)

            # ---- stream matmuls + PSUM->SBUF copies + per-1024-col OUT DMAs
            GRP = 2 * CHUNK
            osb = [constp.tile([P, NHALF], dt.float16, name=f"osb{j}")
                   for j in range(2)]
            g = 0
            for j in range(2):
                for c0 in range(0, NHALF, GRP):
                    gw = min(GRP, NHALF - c0)
                    mm = psm.tile([P, GRP], dt.float32, space="PSUM", tag="mm")
                    for lo in range(0, gw, CHUNK):
                        cw = min(CHUNK, gw - lo)
                        nc.tensor.matmul(out=mm[:, lo:lo + cw],
                                         lhsT=vts[j][:],
                                         rhs=e0t[:, c0 + lo:c0 + lo + cw],
                                         start=True, stop=True)
                    # scalar drains the first 7 groups (vector is still on
                    # the tile-1 head then); afterwards they alternate
                    if g >= 7 and g % 2 == 1:
                        nc.vector.tensor_copy(out=osb[j][:, c0:c0 + gw],
                                              in_=mm[:, :gw])
                    else:
                        nc.scalar.copy(out=osb[j][:, c0:c0 + gw],
                                       in_=mm[:, :gw])
                    if g == 7:
                        # tile-1 transpose emitted mid-stream: the tensor
                        # pipeline never parks waiting on the tile-1 head
                        finish_vt(1)
                    g += 1
                    nc.sync.dma_start(OUT[j * P:(j + 1) * P, c0:c0 + gw],
                                      osb[j][:, c0:c0 + gw])

    nc.compile()
    return nc


def _prep_inputs(inputs):
    x = np.asarray(inputs["x"])
    E0 = np.ascontiguousarray(np.asarray(inputs["E0"], dtype=np.float32))
    E1 = np.asarray(inputs["E1"], dtype=np.float32)
    E2 = np.asarray(inputs["E2"], dtype=np.float32)
    E3 = np.asarray(inputs["E3"], dtype=np.float32)
    E4 = np.asarray(inputs["E4"], dtype=np.float32)
    E5 = np.asarray(inputs["E5"], dtype=np.float32)
    E6 = np.asarray(inputs["E6"], dtype=np.float32)
    rule_C = np.asarray(inputs["rule_C"], dtype=np.float32)
    rule_S = np.asarray(inputs["rule_S"], dtype=np.float32)
    has_rules = np.asarray(inputs["has_rules"])

    idx = np.zeros((B, 4), np.int32)
    idx[:, 0] = x[:, 1]    # r
    idx[:, 1] = x[:, 3]    # t
    idx[:, 2] = x[:, 0]    # h

    def sw(a):
        return np.concatenate([a[:, RANK // 2:], a[:, :RANK // 2]], axis=1)

    rcat = np.ascontiguousarray(np.concatenate(
        [E1, sw(E1), rule_C, sw(rule_C), -rule_S[:, None],
         has_rules.astype(np.float32)[:, None]], axis=1).astype(np.float16))
    tb = np.arange(NTIME) // CYCLE
    TM = E2 + E5[tb]
    TE = E3 + E6[tb]
    tcat = np.ascontiguousarray(np.concatenate(
        [E4, E4, TM, TE, sw(TE)], axis=1).astype(np.float16))
    e0h = E0.astype(np.float16)
    e0g = np.ascontiguousarray(np.concatenate([e0h, e0h], axis=1))
    e0t = np.ascontiguousarray(E0.T).astype(np.float16)
    e0t_halves = [np.ascontiguousarray(e0t[:, :NHALF]),
                  np.ascontiguousarray(e0t[:, NHALF:])]

    in_maps = []
    for c in range(NCORES):
        p = c // 2
        i0 = idx[2 * p * P:(2 * p + 1) * P]        # tile 0 (r,t,h,pad)
        i1 = idx[(2 * p + 1) * P:(2 * p + 2) * P]  # tile 1
        idx2 = np.empty((P, 8), np.int32)
        idx2[:, 0::2] = i0
        idx2[:, 1::2] = i1
        in_maps.append({
            "IDX": np.ascontiguousarray(idx2),
            "E0G": e0g, "RCAT": rcat, "TCAT": tcat,
            "E0T": e0t_halves[c % 2],
        })
    return in_maps


def kernel(**inputs):
    from concourse.bass_utils import run_bass_kernel_spmd

    if "nc" not in _CACHE:
        _CACHE["nc"] = _build()
    nc = _CACHE["nc"]

    in_maps = _prep_inputs(inputs)
    res = run_bass_kernel_spmd(nc, in_maps, core_ids=list(range(NCORES)),
                               trace=TRACE)
    _CACHE["last_result"] = res
    out = np.empty((B, NENT), np.float32)
    for p in range(NCORES // 2):
        lo = res.results[2 * p]["OUT"]        # [256, 0:20000]
        hi = res.results[2 * p + 1]["OUT"]    # [256, 20000:40000]
        rows = slice(2 * p * P, (2 * p + 2) * P)
        out[rows, :NHALF] = lo
        out[rows, NHALF:] = hi
    return out


# revision 29
# speedup vs baseline: 1.0217x; 1.0217x over previous
"""Trainium2 Bass kernel for nn_CTRule (temporal KG scoring model).

Computes, for each of B=1024 queries (h, r, t):
  v = f(E0[h], E1[r], time tables, rule tables)   # [B, 128] elementwise algebra
  scores = v @ E0.T                               # [B, 40000]

Distribution over the 8 NeuronCores (pair-hybrid): the two cores of pair p
process batch tiles 2p, 2p+1 against disjoint halves of the 40000-entity
axis.  Per core: gather (indirect DMA) the per-example table rows, run the
elementwise head in fp16 on VectorE (+GpSimd for the independent rule
chain), transpose v on TensorE, stream this core's E0T half through
40-chunk matmuls per tile, and write the fp16 [256, 20000] block to HBM.

Latency structure (from trace analysis of prior versions):
  * idx is DMA'd by gpsimd itself (lands ~3us; the HWDGE rings only reach
    their first issue slot at ~5-7us after library loads).
  * gathers issue back-to-back on gpsimd right after idx; tile 0's tables
    first.  Tables are host-augmented with swapped halves ([x0|x1]->[x1|x0])
    so every complex-product pair is ONE wide [P,256] fp16 multiply.
  * the head is a single-engine chain on Vector (no cross-engine ping-pong)
    except the rule cmul which GpSimd computes concurrently.
  * E0T chunk loads run on the Activation HWDGE ring from ~7us (no deps).
  * matmul chunks are 512 cols (one PSUM bank); pairs share a [P,1024] PSUM
    tile drained by one copy (Vector/Scalar alternate); every 1024-col group
    is DMA'd to HBM on the Sync ring as soon as its copy lands, so the
    ~31us write stream overlaps everything else.
All head math in fp16 (rel err ~6e-4 total vs the 2e-2 gate).  No
cross-core communication; the host reassembles the 8 blocks.
"""

import numpy as np

P = 128
B = 1024
RANK = 128
NENT = 40000
NREL = 230
NTIME = 365
CYCLE = 120
NCORES = 8
NHALF = NENT // 2        # entity columns per core = 20000
CHUNK = 512              # matmul chunk columns (= one PSUM bank of f32)
LOADCH = 2500            # E0T load-chunk columns (8 loads of 0.64MB)
OUTCH = 1024             # output DMA group columns (= one copy group)

RC_W = 6 * RANK          # [E1 | E1sw | rule_C | rule_Csw | -rS*E1 | hrw] = 768
TC_W = 5 * RANK          # [E4 | E4 | TM | TE | TEsw] = 640
LH_W = 2 * RANK          # [E0row | E0row-swapped] = 256

TRACE = False            # set by test harness for profiling runs
_CACHE = {}


def _build():
    import concourse.bass as bass
    import concourse.mybir as mybir
    import concourse.tile as tile
    from concourse import bacc
    from concourse.masks import make_identity

    dt = mybir.dt
    mult = mybir.AluOpType.mult
    add = mybir.AluOpType.add
    sub = mybir.AluOpType.subtract

    nc = bacc.Bacc("TRN2", target_bir_lowering=False, debug=False,
                   num_devices=NCORES)

    IDX = nc.dram_tensor("IDX", [P, 8], dt.int32, kind="ExternalInput").ap()
    E0G = nc.dram_tensor("E0G", [NENT, LH_W], dt.float16, kind="ExternalInput").ap()
    RCAT = nc.dram_tensor("RCAT", [NREL, RC_W], dt.float16, kind="ExternalInput").ap()
    TCAT = nc.dram_tensor("TCAT", [NTIME, TC_W], dt.float16, kind="ExternalInput").ap()
    E0T = nc.dram_tensor("E0T", [RANK, NHALF], dt.float16, kind="ExternalInput").ap()
    OUT = nc.dram_tensor("OUT", [2 * P, NHALF], dt.float16, kind="ExternalOutput").ap()

    with tile.TileContext(nc) as tc:
        with (
            tc.tile_pool(name="const", bufs=1) as constp,
            tc.tile_pool(name="gath", bufs=1) as gp,
            tc.tile_pool(name="ew", bufs=1) as ew,
            tc.tile_pool(name="pst", bufs=1, space="PSUM") as pst,
            tc.tile_pool(name="psm", bufs=3, space="PSUM") as psm,
        ):
            # ---- idx as the scalar ring's first issue (~7.5us packets; the
            # gpsimd SWDGE path would take ~10.5us for the same load).
            idxt = gp.tile([P, 8], dt.int32)
            nc.scalar.dma_start(idxt[:], IDX[:])

            # idx layout: cols (r0,r1, t0,t1, h0,h1, pad,pad); tile0 first
            lhsv = gp.tile([P, 2, LH_W], dt.float16, name="lhs")
            r8v = gp.tile([P, 2, RC_W], dt.float16, name="r8")
            t8v = gp.tile([P, 2, TC_W], dt.float16, name="t8")
            for j in range(2):
                for dst, src, col in ((r8v, RCAT, 0), (t8v, TCAT, 2),
                                      (lhsv, E0G, 4)):
                    nc.gpsimd.indirect_dma_start(
                        out=dst[:, j, :], out_offset=None, in_=src[:],
                        in_offset=bass.IndirectOffsetOnAxis(
                            ap=idxt[:, col + j:col + j + 1], axis=0))

            # ---- E0T half-table stream on the Activation HWDGE ring
            e0t = constp.tile([RANK, NHALF], dt.float16)
            for c0 in range(0, NHALF, LOADCH):
                nc.scalar.dma_start(e0t[:, c0:c0 + LOADCH],
                                    E0T[:, c0:c0 + LOADCH])

            ident = constp.tile([P, P], dt.float16)
            make_identity(nc, ident[:])

            def VTT(out, a, b_, op):
                nc.vector.tensor_tensor(out=out, in0=a, in1=b_, op=op)

            def GTT(out, a, b_, op):
                nc.gpsimd.tensor_tensor(out=out, in0=a, in1=b_, op=op)

            A = [ew.tile([P, RANK], dt.float16, name=f"A{j}") for j in range(2)]
            Bt = [ew.tile([P, RANK], dt.float16, name=f"B{j}") for j in range(2)]
            PA = [ew.tile([P, 2 * RANK], dt.float16, name=f"PA{j}") for j in range(2)]
            PB = [ew.tile([P, 2 * RANK], dt.float16, name=f"PB{j}") for j in range(2)]
            QQ = [ew.tile([P, 2 * RANK], dt.float16, name=f"QQ{j}") for j in range(2)]
            SS = [ew.tile([P, 2 * RANK], dt.float16, name=f"SS{j}") for j in range(2)]
            DD = [ew.tile([P, 2 * RANK], dt.float16, name=f"DD{j}") for j in range(2)]
            PL = [ew.tile([P, 2 * RANK], dt.float16, name=f"PL{j}") for j in range(2)]
            PT = [ew.tile([P, 2 * RANK], dt.float16, name=f"PT{j}") for j in range(2)]
            t0 = [ew.tile([P, 64], dt.float16, name=f"t0_{j}") for j in range(2)]
            t1 = [ew.tile([P, 64], dt.float16, name=f"t1_{j}") for j in range(2)]
            V = [ew.tile([P, RANK], dt.float16, name=f"V{j}") for j in range(2)]

            def head_rule(j, TT):
                # A = cmul(CT, RC) - rule_S*rel  (host table NRSREL = -rS*E1)
                # PA = [CT|CT]*[RC|RCsw] = [CT0RC0|CT1RC1 | CT0RC1|CT1RC0]
                r8 = r8v[:, j, :]
                TT(PA[j][:], t8v[:, j, 0:256], r8[:, 256:512], mult)
                TT(A[j][:, 0:64], PA[j][:, 0:64], PA[j][:, 64:128], sub)
                TT(A[j][:, 64:128], PA[j][:, 128:192], PA[j][:, 192:256], add)
                TT(A[j][:], A[j][:], r8[:, 512:640], add)

            def head_main(j, TT):
                r8 = r8v[:, j, :]
                t8 = t8v[:, j, :]
                lhs = lhsv[:, j, :]
                RELRELSW = r8[:, 0:256]
                REL = r8[:, 0:128]
                HRW = r8[:, 640:768]
                CT = t8[:, 0:128]
                TM = t8[:, 256:384]
                TM0 = t8[:, 256:320]
                TM1 = t8[:, 320:384]
                TESW2 = t8[:, 384:640]
                LHS = lhs[:, 0:128]
                Aj, Bj = A[j], Bt[j]
                # B = lhs + cmul(REL, LHS):
                # PB = [REL|RELsw]*[L|L] = [RL0L0|RL1L1 | RL1L0|RL0L1]
                TT(PB[j][:], RELRELSW, lhs[:, 0:256], mult)
                TT(Bj[:, 0:64], PB[j][:, 0:64], PB[j][:, 64:128], sub)
                TT(Bj[:, 64:128], PB[j][:, 128:192], PB[j][:, 192:256], add)
                TT(Bj[:], Bj[:], LHS, add)
                # A = rule_score = B + HRW*(A - B);  qq = [A+CT | A+CT]
                TT(Aj[:], Aj[:], Bj[:], sub)
                TT(Aj[:], Aj[:], HRW, mult)
                TT(Aj[:], Aj[:], Bj[:], add)
                TT(QQ[j][:, 0:128], Aj[:], CT, add)
                TT(QQ[j][:, 128:256], Aj[:], CT, add)
                # C = rel_ = REL + complex_mul(REL, q)
                # PC = [REL|RELsw]*[q|q] = [RL0q0|RL1q1 | RL1q0|RL0q1]
                PC = PB[j]
                TT(PC[:], RELRELSW, QQ[j][:], mult)
                TT(Bj[:, 0:64], PC[:, 0:64], PC[:, 64:128], add)
                TT(Bj[:, 64:128], PC[:, 192:256], PC[:, 128:192], sub)
                TT(Bj[:], Bj[:], REL, add)
                # SS = [S|Ssw], DD = [D|D]; S = rel_+time, D = rel_-time
                TT(SS[j][:, 0:128], Bj[:], TM, add)
                TT(SS[j][:, 128:192], Bj[:, 64:128], TM1, add)
                TT(SS[j][:, 192:256], Bj[:, 0:64], TM0, add)
                TT(DD[j][:, 0:128], Bj[:], TM, sub)
                TT(DD[j][:, 128:256], Bj[:], TM, sub)
                # PL = [L|L]*[S|Ssw] = [L0S0|L1S1 | L0S1|L1S0]
                # PT = [TE|TEsw]*[D|D] = [TE0D0|TE1D1 | TE1D0|TE0D1]
                TT(PL[j][:], lhs[:, 0:256], SS[j][:], mult)
                TT(PT[j][:], TESW2, DD[j][:], mult)
                # V0 = (L0S0 - L1S1) + (TE0D0 + TE1D1)
                TT(t0[j][:], PL[j][:, 0:64], PL[j][:, 64:128], sub)
                TT(t1[j][:], PT[j][:, 0:64], PT[j][:, 64:128], add)
                TT(V[j][:, 0:64], t0[j][:], t1[j][:], add)
                # V1 = (L0S1 + L1S0) + (TE1D0 - TE0D1)
                TT(t0[j][:], PL[j][:, 128:192], PL[j][:, 192:256], add)
                TT(t1[j][:], PT[j][:, 128:192], PT[j][:, 192:256], sub)
                TT(V[j][:, 64:128], t0[j][:], t1[j][:], add)

            vts = []

            def finish_vt(j):
                vt_ps = pst.tile([P, P], dt.float16, space="PSUM", tag="vtps")
                nc.tensor.transpose(out=vt_ps[:], in_=V[j][:], identity=ident[:])
                vt = constp.tile([P, P], dt.float16, name=f"vt{j}")
                nc.scalar.copy(out=vt[:], in_=vt_ps[:])
                vts.append(vt)

            head_rule(0, GTT)
            head_main(0, VTT)
            finish_vt(0)
            head_rule(1, GTT)
            head_main(1, GTT)

            # ---- stream matmuls + PSUM->SBUF copies + per-1024-col OUT DMAs
            GRP = 2 * CHUNK
            osb = [constp.tile([P, NHALF], dt.float16, name=f"osb{j}")
                   for j in range(2)]
            g = 0
            for j in range(2):
                for c0 in range(0, NHALF, GRP):
                    gw = min(GRP, NHALF - c0)
                    mm = psm.tile([P, GRP], dt.float32, space="PSUM", tag="mm")
                    for lo in range(0, gw, CHUNK):
                        cw = min(CHUNK, gw - lo)
                        nc.tensor.matmul(out=mm[:, lo:lo + cw],
                                         lhsT=vts[j][:],
                                         rhs=e0t[:, c0 + lo:c0 + lo + cw],
                                         start=True, stop=True)
                    if g % 2 == 0:
                        nc.vector.tensor_copy(out=osb[j][:, c0:c0 + gw],
                                              in_=mm[:, :gw])
                    else:
                        nc.scalar.copy(out=osb[j][:, c0:c0 + gw],
                                       in_=mm[:, :gw])
                    if g == 7:
                        # tile-1 transpose emitted mid-stream: the tensor
                        # pipeline never parks waiting on the tile-1 head
                        finish_vt(1)
                    g += 1
                    nc.sync.dma_start(OUT[j * P:(j + 1) * P, c0:c0 + gw],
                                      osb[j][:, c0:c0 + gw])

    nc.compile()
    return nc


def _prep_inputs(inputs):
    x = np.asarray(inputs["x"])
    E0 = np.ascontiguousarray(np.asarray(inputs["E0"], dtype=np.float32))
    E1 = np.asarray(inputs["E1"], dtype=np.float32)
    E2 = np.asarray(inputs["E2"], dtype=np.float32)
    E3 = np.asarray(inputs["E3"], dtype=np.float32)
    E4 = np.asarray(inputs["E4"], dtype=np.float32)
    E5 = np.asarray(inputs["E5"], dtype=np.float32)
    E6 = np.asarray(inputs["E6"], dtype=np.float32)
    rule_C = np.asarray(inputs["rule_C"], dtype=np.float32)
    rule_S = np.asarray(inputs["rule_S"], dtype=np.float32)
    has_rules = np.asarray(inputs["has_rules"])

    idx = np.zeros((B, 4), np.int32)
    idx[:, 0] = x[:, 1]    # r
    idx[:, 1] = x[:, 3]    # t
    idx[:, 2] = x[:, 0]    # h

    def sw(a):
        return np.concatenate([a[:, RANK // 2:], a[:, :RANK // 2]], axis=1)

    hrw = np.repeat(has_rules.astype(np.float32)[:, None], RANK, axis=1)
    rcat = np.ascontiguousarray(np.concatenate(
        [E1, sw(E1), rule_C, sw(rule_C), -rule_S[:, None] * E1,
         hrw], axis=1).astype(np.float16))
    tb = np.arange(NTIME) // CYCLE
    TM = E2 + E5[tb]
    TE = E3 + E6[tb]
    tcat = np.ascontiguousarray(np.concatenate(
        [E4, E4, TM, TE, sw(TE)], axis=1).astype(np.float16))
    e0h = E0.astype(np.float16)
    e0g = np.ascontiguousarray(np.concatenate([e0h, e0h], axis=1))
    e0t = np.ascontiguousarray(E0.T).astype(np.float16)
    e0t_halves = [np.ascontiguousarray(e0t[:, :NHALF]),
                  np.ascontiguousarray(e0t[:, NHALF:])]

    in_maps = []
    for c in range(NCORES):
        p = c // 2
        i0 = idx[2 * p * P:(2 * p + 1) * P]        # tile 0 (r,t,h,pad)
        i1 = idx[(2 * p + 1) * P:(2 * p + 2) * P]  # tile 1
        idx2 = np.empty((P, 8), np.int32)
        idx2[:, 0::2] = i0
        idx2[:, 1::2] = i1
        in_maps.append({
            "IDX": np.ascontiguousarray(idx2),
            "E0G": e0g, "RCAT": rcat, "TCAT": tcat,
            "E0T": e0t_halves[c % 2],
        })
    return in_maps


def kernel(**inputs):
    from concourse.bass_utils import run_bass_kernel_spmd

    if "nc" not in _CACHE:
        _CACHE["nc"] = _build()
    nc = _CACHE["nc"]

    in_maps = _prep_inputs(inputs)
    res = run_bass_kernel_spmd(nc, in_maps, core_ids=list(range(NCORES)),
                               trace=TRACE)
    _CACHE["last_result"] = res
    out = np.empty((B, NENT), np.float32)
    for p in range(NCORES // 2):
        lo = res.results[2 * p]["OUT"]        # [256, 0:20000]
        hi = res.results[2 * p + 1]["OUT"]    # [256, 20000:40000]
        rows = slice(2 * p * P, (2 * p + 2) * P)
        out[rows, :NHALF] = lo
        out[rows, NHALF:] = hi
    return out
